# revision 5
# baseline (speedup 1.0000x reference)
"""Trainium2 Bass kernel for MultiInnerProductDecoder (DistMult edge scoring).

score_e = sigmoid( sum_d z[src_e, d] * z[dst_e, d] * weight[type_e, d] )

Sharding: edges split evenly across 8 NeuronCores (data parallel over E);
z and weight replicated on every core.

Per-core strategy: SWDGE dma_gather (InstDMAGatherAnt) does the per-edge row
gathers — thousands of 512B rows per instruction. Its indices are int16, so
the 100k-row z table is addressed as 4 range-subtables of 25000 rows; the
host groups each core's edges by (src//25000, dst//25000) into 16 groups so
every chunk's src/dst gathers hit a single subtable. Groups are padded to a
fixed capacity with dummy edges (index 0) to keep all shapes compile-time
static; the host applies the inverse permutation to the returned scores.
"""

import numpy as np

# Problem constants (hardcoded per harness contract — no spec.json reads).
N_DRUGS = 100000
NUM_ET = 1000
IN_DIM = 128   # D
N_EDGES = 2000000
N_CORES = 8
E_PER_CORE = N_EDGES // N_CORES   # 250000

SPLIT = 4                  # z subtables
RANGE = N_DRUGS // SPLIT   # 25000 rows per subtable (int16-addressable)
NG = SPLIT * SPLIT         # 16 groups keyed by (src block, dst block)
G_CAP = 16384              # padded per-group capacity (max observed ~15.9k)
CHUNK = 4096               # edges per gather chunk
CPG = G_CAP // CHUNK       # 4 chunks per group
NCHUNK = NG * CPG          # 64 chunks per core
E_PAD = NCHUNK * CHUNK     # 262144 padded edges per core

P = 128
K = CHUNK // P             # 32 rows per partition per chunk
S = CHUNK // 16            # 256 idx columns in the wrapped int16 layout

_cache = {}


def _build_nc():
    import concourse.bass as bass
    import concourse.tile as tile
    from concourse import bacc, mybir

    f32 = mybir.dt.float32
    i16 = mybir.dt.int16
    D = IN_DIM

    # Bacc (not plain Bass): its compile() pipeline runs
    # generate_event_semaphores, which splits multi-sem waits into
    # EventSemaphore instructions (TRN2 allows at most 1 wait per inst).
    nc = bacc.Bacc(None)
    z_ext = nc.declare_dram_parameter("z", [N_DRUGS, D], f32, isOutput=False)
    w_ext = nc.declare_dram_parameter("weight", [NUM_ET, D], f32, isOutput=False)
    # Pre-wrapped int16 gather indices: [chunk, 128, S]; logical index j of a
    # chunk sits at partition j%16, column j//16 (replicated 8x over
    # partitions 16..127).
    sidx = nc.declare_dram_parameter("sidx", [NCHUNK, P, S], i16, isOutput=False)
    didx = nc.declare_dram_parameter("didx", [NCHUNK, P, S], i16, isOutput=False)
    tidx = nc.declare_dram_parameter("tidx", [NCHUNK, P, S], i16, isOutput=False)
    out_ext = nc.declare_dram_parameter("out", [E_PAD], f32, isOutput=True)

    # Chunk c's scores land at out[c*CHUNK + p*K + k] from score tile [p, k];
    # gathered edge j of the chunk lives at (p, k) = (j % 128, j // 128).
    outv = out_ext[:].rearrange("(c p k) -> c p k", p=P, k=K)

    with tile.TileContext(nc) as tc:
        with (
            tc.tile_pool(name="idx", bufs=3) as idxp,
            tc.tile_pool(name="gsrc", bufs=2) as psrc,
            tc.tile_pool(name="gdst", bufs=2) as pdst,
            tc.tile_pool(name="gw", bufs=2) as pw,
            tc.tile_pool(name="score", bufs=3) as scp,
        ):
            for c in range(NCHUNK):
                g = c // CPG
                r, s = g // SPLIT, g % SPLIT

                si = idxp.tile([P, S], i16, tag="si")
                nc.sync.dma_start(out=si[:], in_=sidx[c])
                di = idxp.tile([P, S], i16, tag="di")
                nc.sync.dma_start(out=di[:], in_=didx[c])
                ti = idxp.tile([P, S], i16, tag="ti")
                nc.sync.dma_start(out=ti[:], in_=tidx[c])

                zs = psrc.tile([P, K * D], f32, tag="zs")
                nc.gpsimd.dma_gather(
                    out_ap=zs[:].rearrange("p (k d) -> p k d", d=D),
                    in_ap=z_ext[r * RANGE : (r + 1) * RANGE, :],
                    idxs_ap=si[:],
                    num_idxs=CHUNK,
                    num_idxs_reg=CHUNK,
                    elem_size=D,
                )
                zd = pdst.tile([P, K * D], f32, tag="zd")
                nc.gpsimd.dma_gather(
                    out_ap=zd[:].rearrange("p (k d) -> p k d", d=D),
                    in_ap=z_ext[s * RANGE : (s + 1) * RANGE, :],
                    idxs_ap=di[:],
                    num_idxs=CHUNK,
                    num_idxs_reg=CHUNK,
                    elem_size=D,
                )
                wt = pw.tile([P, K * D], f32, tag="wt")
                nc.gpsimd.dma_gather(
                    out_ap=wt[:].rearrange("p (k d) -> p k d", d=D),
                    in_ap=w_ext[:, :],
                    idxs_ap=ti[:],
                    num_idxs=CHUNK,
                    num_idxs_reg=CHUNK,
                    elem_size=D,
                )

                nc.vector.tensor_mul(zs[:], zs[:], zd[:])
                nc.vector.tensor_mul(zs[:], zs[:], wt[:])

                sc = scp.tile([P, K], f32, tag="sc")
                nc.vector.reduce_sum(
                    out=sc[:],
                    in_=zs[:].rearrange("p (k d) -> p k d", d=D),
                    axis=mybir.AxisListType.X,
                )
                sg = scp.tile([P, K], f32, tag="sg")
                nc.scalar.activation(
                    out=sg[:], in_=sc[:],
                    func=mybir.ActivationFunctionType.Sigmoid,
                )
                nc.sync.dma_start(out=outv[c], in_=sg[:])

    nc.compile()
    return nc


def _wrap_idx(arr):
    """[E_PAD] int16 -> [NCHUNK, 128, S]: chunk-local index j at partition
    j%16, column j//16, replicated 8x across the partition dim."""
    a = arr.reshape(NCHUNK, S, 16).transpose(0, 2, 1)  # [c, 16, S]
    return np.ascontiguousarray(np.tile(a, (1, 8, 1)))


def _shard_inputs(z, edge_index, edge_type, weight):
    z = np.ascontiguousarray(np.asarray(z, dtype=np.float32))
    weight = np.ascontiguousarray(np.asarray(weight, dtype=np.float32))
    edge_index = np.asarray(edge_index)
    edge_type = np.asarray(edge_type)

    in_maps = []
    gathers = []  # per core: (order, outpos) to unscramble scores
    for c in range(N_CORES):
        lo, hi = c * E_PER_CORE, (c + 1) * E_PER_CORE
        src = np.asarray(edge_index[0, lo:hi], dtype=np.int64)
        dst = np.asarray(edge_index[1, lo:hi], dtype=np.int64)
        et = np.asarray(edge_type[lo:hi], dtype=np.int64)

        rb, sb = src // RANGE, dst // RANGE
        g = rb * SPLIT + sb
        counts = np.bincount(g, minlength=NG)
        if counts.max() > G_CAP:
            raise RuntimeError(f"group overflow: {counts.max()} > {G_CAP}")
        order = np.argsort(g, kind="stable")  # original edge ids, group-major

        # padded stream position q for each (sorted) edge
        starts = np.cumsum(counts) - counts
        jj = np.arange(E_PER_CORE) - np.repeat(starts, counts)
        q = np.repeat(np.arange(NG), counts) * G_CAP + jj

        s16 = np.zeros(E_PAD, np.int16)
        d16 = np.zeros(E_PAD, np.int16)
        t16 = np.zeros(E_PAD, np.int16)
        s16[q] = (src - rb * RANGE)[order].astype(np.int16)
        d16[q] = (dst - sb * RANGE)[order].astype(np.int16)
        t16[q] = et[order].astype(np.int16)

        # where each edge's score lands in the kernel's output array
        ch, r = q // CHUNK, q % CHUNK
        outpos = ch * CHUNK + (r % P) * K + (r // P)

        in_maps.append({
            "z": z, "weight": weight,
            "sidx": _wrap_idx(s16), "didx": _wrap_idx(d16),
            "tidx": _wrap_idx(t16),
        })
        gathers.append((order, outpos))
    return in_maps, gathers


def run(z, edge_index, edge_type, weight, trace=False, **trace_kw):
    """Returns (output [N_EDGES] f32, BassKernelResults)."""
    from concourse.bass_utils import run_bass_kernel_spmd

    if "nc" not in _cache:
        _cache["nc"] = _build_nc()
    nc = _cache["nc"]

    in_maps, gathers = _shard_inputs(z, edge_index, edge_type, weight)
    res = run_bass_kernel_spmd(
        nc, in_maps, core_ids=list(range(N_CORES)), trace=trace, **trace_kw
    )
    out = np.empty(N_EDGES, np.float32)
    for c in range(N_CORES):
        order, outpos = gathers[c]
        piece = np.empty(E_PER_CORE, np.float32)
        piece[order] = res.results[c]["out"][outpos]
        out[c * E_PER_CORE : (c + 1) * E_PER_CORE] = piece
    return out, res


def kernel(z, edge_index, edge_type, weight):
    out, _ = run(z, edge_index, edge_type, weight)
    return out
